# revision 5
# baseline (speedup 1.0000x reference)
"""2-layer multi-head GAT on 8 TRN2 NeuronCores (Bass/Tile).

Sharding: destination-node blocks. Core i owns nodes [i*NPC, (i+1)*NPC) and
all edges whose dst lands there, so edge softmax + aggregation are fully
core-local (no all-reduce). Node features / z-tables are replicated via two
small AllGathers between the dense phases.

Edge phase per layer: dma_gather of per-src table rows (z | s_src packed),
dma_gather of per-dst score rows from a core-local table, numerator
exp(leakyrelu(s_src+s_dst)) on DVE/ACT, then dma_scatter_add of
[numer*z | numer] into HBM accumulators. Scatter calls are built host-side
so each call has at most one edge per destination row (the SDMA CCE
read-modify-write loses updates for any duplicate index in flight), with
two accumulators alternating between calls.
"""
import sys
sys.path.insert(0, "/opt/trn_rl_repo")

import numpy as np
import ml_dtypes

import concourse.bass as bass
import concourse.bacc as bacc
import concourse.tile as tile
import concourse.mybir as mybir
from concourse.bass_utils import run_bass_kernel_spmd
from concourse.masks import make_identity

F32 = mybir.dt.float32
BF16 = mybir.dt.bfloat16
I16 = mybir.dt.int16

NCORES = 8
HALF = 32768           # int16 gather index split
NEG = -1.0e30          # dst-score of dummy rows -> numerator exactly 0
PIECE_CHUNKS = 16      # max 128-chunks per scatter call (SWDGE ring capacity)


# ----------------------------------------------------------------- host prep

def _round_up(x, m):
    return (x + m - 1) // m * m


def _plan_groups(dstl, gsrc, n_groups, npc):
    """Assign each edge to a group such that no two edges in a group share
    dstl. Returns group id per edge and rank-within-node (for determinism).
    """
    order = np.argsort(dstl, kind="stable")
    ds = dstl[order]
    run_start = np.zeros(len(ds), dtype=np.int64)
    if len(ds):
        new = np.ones(len(ds), dtype=bool)
        new[1:] = ds[1:] != ds[:-1]
        starts = np.flatnonzero(new)
        lens = np.diff(np.append(starts, len(ds)))
        run_start = np.repeat(starts, lens)
    rank_sorted = np.arange(len(ds)) - run_start
    start = (ds.astype(np.int64) * 2654435761) % n_groups
    grp_sorted = (start + rank_sorted) % n_groups
    grp = np.empty(len(ds), dtype=np.int64)
    grp[order] = grp_sorted
    return grp


def preprocess(h, src, dst, W1, a1, W2, a2):
    N, IN_DIM = h.shape
    HEADS, _, HID = W1.shape
    OUT = W2.shape[1]
    E = src.shape[0]
    npc = N // NCORES
    rows = _round_up(npc + 1, 128)
    dummy = npc

    # weight folding (weights-only algebra)
    w1cat = np.transpose(W1, (1, 0, 2)).reshape(IN_DIM, HEADS * HID)
    w1s = np.stack([W1[hh] @ a1[hh, :HID] for hh in range(HEADS)], 1)   # [IN,H]
    w1d = np.stack([W1[hh] @ a1[hh, HID:] for hh in range(HEADS)], 1)
    wc1 = np.concatenate([w1cat, w1s, w1d], 1).astype(np.float32)       # [IN, 264]
    wc2 = np.concatenate([W2, (W2 @ a2[:OUT])[:, None],
                          (W2 @ a2[OUT:])[:, None]], 1).astype(np.float32)  # [256,66]

    core_of = dst // npc
    gsrc_all = (src // npc) * rows + (src % npc)      # global table row per src

    per_core = []
    maxdeg = 0
    for c in range(NCORES):
        m = core_of == c
        dstl = (dst[m] - c * npc).astype(np.int64)
        gsrc = gsrc_all[m].astype(np.int64)
        deg = np.bincount(dstl, minlength=npc)
        maxdeg = max(maxdeg, int(deg.max()) if len(deg) else 0)
        per_core.append((dstl, gsrc))

    e_max = max(len(d) for d, _ in per_core)
    n_groups = max(int(np.ceil(e_max / 1660.0)), maxdeg + 2)

    # group assignment + per-(core, group, half) counts
    plans = []
    low_cnt = np.zeros((NCORES, n_groups), dtype=np.int64)
    high_cnt = np.zeros((NCORES, n_groups), dtype=np.int64)
    for c in range(NCORES):
        dstl, gsrc = per_core[c]
        grp = _plan_groups(dstl, gsrc, n_groups, npc)
        half = (gsrc >= HALF).astype(np.int64)
        np.add.at(low_cnt[c], grp[half == 0], 1)
        np.add.at(high_cnt[c], grp[half == 1], 1)
        plans.append((dstl, gsrc, grp, half))

    L = np.maximum(1, np.ceil(low_cnt.max(0) / 128.0).astype(np.int64))
    Hc = np.ceil(high_cnt.max(0) / 128.0).astype(np.int64)  # may be 0
    chunks = L + Hc
    bases = np.concatenate([[0], np.cumsum(chunks)]) * 128
    total_pos = int(bases[-1])

    # per-core position-ordered index arrays
    eidx = np.zeros((NCORES, total_pos), dtype=np.int16)
    didx = np.full((NCORES, total_pos), dummy, dtype=np.int16)
    for c in range(NCORES):
        dstl, gsrc, grp, half = plans[c]
        key = grp * 2 + half
        order = np.argsort(key, kind="stable")
        ks = key[order]
        new = np.ones(len(ks), dtype=bool)
        new[1:] = ks[1:] != ks[:-1]
        starts = np.flatnonzero(new)
        lens = np.diff(np.append(starts, len(ks)))
        within = np.arange(len(ks)) - np.repeat(starts, lens)
        g_o = grp[order]
        h_o = half[order]
        pos = bases[g_o] + h_o * L[g_o] * 128 + within
        eidx[c, pos] = (gsrc[order] - h_o * HALF).astype(np.int16)
        didx[c, pos] = dstl[order].astype(np.int16)

    # static group table for the device graph
    groups = [(int(bases[g]), int(L[g]), int(Hc[g])) for g in range(n_groups)]

    struct = dict(
        N=N, E=E, IN_DIM=IN_DIM, HEADS=HEADS, HID=HID, OUT=OUT,
        npc=npc, rows=rows, dummy=dummy, total_pos=total_pos, groups=groups,
    )

    def idx_tile(a):     # linear positions -> [32, n/16] int16, replicated x2
        t = a.reshape(-1, 16).T.copy()
        return np.concatenate([t, t], 0)

    lo = npc - (rows // 128 - 1) * 128
    dmask_host = np.zeros((128, 4), dtype=np.float32)
    dmask_host[lo:, :] = NEG
    in_maps = []
    for c in range(NCORES):
        hs = np.zeros((rows, IN_DIM), dtype=np.float32)
        hs[:npc] = h[c * npc:(c + 1) * npc]
        in_maps.append({
            "h": hs,
            "eidx": idx_tile(eidx[c]),
            "didx": idx_tile(didx[c]),
            "wc1": wc1,
            "wc2": wc2,
            "azero": np.zeros((rows, 384), dtype=ml_dtypes.bfloat16),
            "azero2": np.zeros((rows, 128), dtype=np.float32),
            "dmask": dmask_host,
        })
    return struct, in_maps


# --------------------------------------------------------------- bass graph

def build(s):
    npc, rows, total_pos = s["npc"], s["rows"], s["total_pos"]
    groups = s["groups"]
    IN_DIM, HEADS, HID, OUT = s["IN_DIM"], s["HEADS"], s["HID"], s["OUT"]
    ZC = HEADS * HID              # 256
    NT = rows // 128              # node tiles per core
    n_half_rows = NCORES * rows > HALF

    nc = bacc.Bacc("TRN2", target_bir_lowering=False, debug=False,
                   num_devices=NCORES)

    h_in = nc.dram_tensor("h", [rows, IN_DIM], F32, kind="ExternalInput")
    eidx_in = nc.dram_tensor("eidx", [32, total_pos // 16], I16,
                             kind="ExternalInput")
    didx_in = nc.dram_tensor("didx", [32, total_pos // 16], I16,
                             kind="ExternalInput")
    wc1_in = nc.dram_tensor("wc1", [IN_DIM, ZC + 8], F32, kind="ExternalInput")
    wc2_in = nc.dram_tensor("wc2", [ZC, OUT + 2], F32, kind="ExternalInput")
    az_in = nc.dram_tensor("azero", [rows, 384], BF16, kind="ExternalInput")
    az2_in = nc.dram_tensor("azero2", [rows, 128], F32, kind="ExternalInput")
    dmask_in = nc.dram_tensor("dmask", [128, 4], F32, kind="ExternalInput")
    out_ext = nc.dram_tensor("out", [rows, OUT], F32, kind="ExternalOutput")

    with tile.TileContext(nc) as tc:
        with (
            tc.tile_pool(name="dram", bufs=1, space="DRAM") as dram,
            tc.tile_pool(name="const", bufs=1) as const,
            tc.tile_pool(name="psum_c", bufs=2, space="PSUM") as psum_c,
        ):
            # persistent DRAM
            t1_loc = dram.tile([rows, 384], BF16)
            t1_full = dram.tile([NCORES * rows, 384], BF16)
            t2_loc = dram.tile([rows, 128], F32)
            t2_full = dram.tile([NCORES * rows, 128], F32)
            s_tbl = dram.tile([rows, 128], BF16)
            a1p0 = dram.tile([rows, 384], BF16)
            a1p1 = dram.tile([rows, 384], BF16)
            a2p0 = dram.tile([rows, 128], F32)
            a2p1 = dram.tile([rows, 128], F32)
            a1p = [a1p0, a1p1]
            a2p = [a2p0, a2p1]

            # zero accumulators (re-exec safe)
            for p in range(2):
                nc.sync.dma_start(a1p[p][:], az_in[:])
                nc.sync.dma_start(a2p[p][:], az2_in[:])

            # consts
            ident = const.tile([128, 128], F32)
            make_identity(nc, ident[:])
            wc1_t = const.tile([IN_DIM, ZC + 8], F32)
            nc.sync.dma_start(wc1_t[:], wc1_in[:])
            wc2a = const.tile([128, OUT + 2], F32)
            wc2b = const.tile([128, OUT + 2], F32)
            nc.sync.dma_start(wc2a[:], wc2_in[0:128, :])
            nc.sync.dma_start(wc2b[:], wc2_in[128:256, :])
            dmask = const.tile([128, 4], F32)
            nc.sync.dma_start(dmask[:], dmask_in[:])
            eidx_t = const.tile([32, total_pos // 16], I16)
            didx_t = const.tile([32, total_pos // 16], I16)
            nc.sync.dma_start(eidx_t[:], eidx_in[:])
            nc.sync.dma_start(didx_t[:], didx_in[:])

            # ---------------- D1: z1 | s_src1 | s_dst1, build T1 + S ------
            with tc.tile_pool(name="d1", bufs=3) as d1:
                for t in range(NT):
                    ht = d1.tile([128, IN_DIM], F32, tag="ht")
                    nc.sync.dma_start(ht[:], h_in[t * 128:(t + 1) * 128, :])
                    hT_ps = psum_c.tile([128, 128], F32, tag="tp")
                    nc.tensor.transpose(hT_ps[:], ht[:], ident[:])
                    hT = d1.tile([128, 128], F32, tag="hT")
                    nc.vector.tensor_copy(hT[:], hT_ps[:])
                    zps = psum_c.tile([128, ZC + 8], F32, tag="zp")
                    nc.tensor.matmul(zps[:], hT[:], wc1_t[:])

                    t1t = d1.tile([128, 384], BF16, tag="t1t")
                    nc.vector.tensor_copy(t1t[:, 0:ZC], zps[:, 0:ZC])
                    nc.vector.tensor_copy(
                        t1t[:, ZC:ZC + 16].bitcast(F32), zps[:, ZC:ZC + 8])
                    st = d1.tile([128, 128], BF16, tag="st")
                    if t == NT - 1:
                        # dummy + pad rows: dst-score += -1e30 (numer -> 0)
                        nc.vector.tensor_add(
                            st[:, 0:8].bitcast(F32),
                            zps[:, ZC + 4:ZC + 8], dmask[:])
                    else:
                        nc.vector.tensor_copy(
                            st[:, 0:8].bitcast(F32), zps[:, ZC + 4:ZC + 8])
                    nc.sync.dma_start(
                        t1_loc[t * 128:(t + 1) * 128, :], t1t[:])
                    nc.sync.dma_start(
                        s_tbl[t * 128:(t + 1) * 128, :], st[:])

            nc.gpsimd.collective_compute(
                "AllGather", mybir.AluOpType.bypass,
                replica_groups=[list(range(NCORES))],
                ins=[t1_loc.opt()], outs=[t1_full.opt()],
            )

            # ---------------- L1 edge phase -------------------------------
            with (
                tc.tile_pool(name="l1", bufs=3) as l1,
                tc.tile_pool(name="l1m", bufs=3) as l1m,
            ):
                for gi, (base, Lg, Hg) in enumerate(groups):
                    C = Lg + Hg
                    g = l1.tile([128, C, 384], BF16, tag="g")
                    nc.gpsimd.dma_gather(
                        g[:, 0:Lg, :], t1_full[:],
                        eidx_t[:, base // 16:(base + Lg * 128) // 16],
                        num_idxs=Lg * 128, num_idxs_reg=Lg * 128,
                        elem_size=384, single_packet=False)
                    if Hg:
                        b2 = base + Lg * 128
                        nc.gpsimd.dma_gather(
                            g[:, Lg:C, :], t1_full[HALF:, :],
                            eidx_t[:, b2 // 16:(b2 + Hg * 128) // 16],
                            num_idxs=Hg * 128, num_idxs_reg=Hg * 128,
                            elem_size=384, single_packet=False)
                    d = l1.tile([128, C, 128], BF16, tag="d")
                    nc.gpsimd.dma_gather(
                        d[:, :, :], s_tbl[:],
                        didx_t[:, base // 16:(base + C * 128) // 16],
                        num_idxs=C * 128, num_idxs_reg=C * 128,
                        elem_size=128, single_packet=False)

                    q = l1.tile([128, C, HEADS], F32, tag="q")
                    nc.vector.tensor_add(
                        q[:], g[:, :, ZC:ZC + 8].bitcast(F32),
                        d[:, :, 0:8].bitcast(F32))
                    q2 = l1.tile([128, C, HEADS], F32, tag="q2")
                    nc.vector.tensor_scalar_mul(q2[:], q[:], 0.01)
                    nc.vector.tensor_max(q2[:], q2[:], q[:])
                    num = l1.tile([128, C, HEADS], F32, tag="num")
                    nc.scalar.activation(
                        num[:], q2[:], mybir.ActivationFunctionType.Exp)

                    m = l1m.tile([128, C, ZC + HEADS], BF16, tag="m")
                    nc.vector.tensor_tensor(
                        m[:, :, 0:ZC].rearrange(
                            "p c (h x) -> p c h x", h=HEADS),
                        g[:, :, 0:ZC].rearrange(
                            "p c (h x) -> p c h x", h=HEADS),
                        num[:, :, :, None].to_broadcast((128, C, HEADS, HID)),
                        mybir.AluOpType.mult)
                    nc.vector.tensor_copy(m[:, :, ZC:ZC + HEADS], num[:])

                    acc = a1p[gi % 2]
                    for c0 in range(0, C, PIECE_CHUNKS):
                        cn = min(PIECE_CHUNKS, C - c0)
                        p0 = base + c0 * 128
                        nc.gpsimd.dma_scatter_add(
                            acc[:, 0:ZC + HEADS], m[:, c0:c0 + cn, :],
                            didx_t[:, p0 // 16:(p0 + cn * 128) // 16],
                            num_idxs=cn * 128, num_idxs_reg=cn * 128,
                            elem_size=ZC + HEADS, elem_step=384)

            # ---------------- D2: h1 = elu(agg/numer); z2 | s2 ------------
            with tc.tile_pool(name="d2", bufs=3) as d2:
                for t in range(NT):
                    r0, r1 = t * 128, (t + 1) * 128
                    a0 = d2.tile([128, ZC + HEADS], BF16, tag="a0")
                    a1t = d2.tile([128, ZC + HEADS], BF16, tag="a1")
                    nc.sync.dma_start(a0[:], a1p[0][r0:r1, 0:ZC + HEADS])
                    nc.sync.dma_start(a1t[:], a1p[1][r0:r1, 0:ZC + HEADS])
                    msum = d2.tile([128, ZC + HEADS], F32, tag="msum")
                    nc.vector.tensor_add(msum[:], a0[:], a1t[:])
                    nm = d2.tile([128, HEADS], F32, tag="nm")
                    nc.vector.tensor_scalar_max(
                        nm[:], msum[:, ZC:ZC + HEADS], 1e-30)
                    rec = d2.tile([128, HEADS], F32, tag="rec")
                    nc.vector.reciprocal(rec[:], nm[:])
                    h1 = d2.tile([128, ZC], F32, tag="h1")
                    nc.vector.tensor_tensor(
                        h1[:].rearrange("p (h x) -> p h x", h=HEADS),
                        msum[:, 0:ZC].rearrange("p (h x) -> p h x", h=HEADS),
                        rec[:, :, None].to_broadcast((128, HEADS, HID)),
                        mybir.AluOpType.mult)
                    # elu = relu(x) + exp(min(x,0)) - 1
                    relu = d2.tile([128, ZC], F32, tag="relu")
                    nc.scalar.activation(
                        relu[:], h1[:], mybir.ActivationFunctionType.Relu)
                    xm = d2.tile([128, ZC], F32, tag="xm")
                    nc.vector.tensor_scalar_min(xm[:], h1[:], 0.0)
                    ex = d2.tile([128, ZC], F32, tag="ex")
                    nc.scalar.activation(
                        ex[:], xm[:], mybir.ActivationFunctionType.Exp)
                    h1e = d2.tile([128, ZC], F32, tag="h1e")
                    nc.vector.tensor_add(h1e[:], relu[:], ex[:])
                    nc.vector.tensor_scalar_add(h1e[:], h1e[:], -1.0)

                    z2ps = psum_c.tile([128, OUT + 2], F32, tag="z2p")
                    for kk in range(2):
                        tp = psum_c.tile([128, 128], F32, tag="tp")
                        nc.tensor.transpose(
                            tp[:], h1e[:, kk * 128:(kk + 1) * 128], ident[:])
                        hT2 = d2.tile([128, 128], F32, tag="hT2")
                        nc.vector.tensor_copy(hT2[:], tp[:])
                        nc.tensor.matmul(
                            z2ps[:], hT2[:], wc2a[:] if kk == 0 else wc2b[:],
                            start=(kk == 0), stop=(kk == 1))

                    t2t = d2.tile([128, 128], F32, tag="t2t")
                    nc.vector.tensor_copy(t2t[:, 0:OUT + 1], z2ps[:, 0:OUT + 1])
                    st2 = d2.tile([128, 2], BF16, tag="st2")
                    if t == NT - 1:
                        nc.vector.tensor_add(
                            st2[:].bitcast(F32),
                            z2ps[:, OUT + 1:OUT + 2], dmask[:, 0:1])
                    else:
                        nc.vector.tensor_copy(
                            st2[:].bitcast(F32), z2ps[:, OUT + 1:OUT + 2])
                    nc.sync.dma_start(t2_loc[r0:r1, :], t2t[:])
                    nc.sync.dma_start(s_tbl[r0:r1, 8:10], st2[:])

            nc.gpsimd.collective_compute(
                "AllGather", mybir.AluOpType.bypass,
                replica_groups=[list(range(NCORES))],
                ins=[t2_loc.opt()], outs=[t2_full.opt()],
            )

            # ---------------- L2 edge phase -------------------------------
            with (
                tc.tile_pool(name="l2", bufs=3) as l2,
                tc.tile_pool(name="l2m", bufs=3) as l2m,
            ):
                for gi, (base, Lg, Hg) in enumerate(groups):
                    C = Lg + Hg
                    g = l2.tile([128, C, 128], F32, tag="g2")
                    nc.gpsimd.dma_gather(
                        g[:, 0:Lg, :], t2_full[:],
                        eidx_t[:, base // 16:(base + Lg * 128) // 16],
                        num_idxs=Lg * 128, num_idxs_reg=Lg * 128,
                        elem_size=128, single_packet=False)
                    if Hg:
                        b2 = base + Lg * 128
                        nc.gpsimd.dma_gather(
                            g[:, Lg:C, :], t2_full[HALF:, :],
                            eidx_t[:, b2 // 16:(b2 + Hg * 128) // 16],
                            num_idxs=Hg * 128, num_idxs_reg=Hg * 128,
                            elem_size=128, single_packet=False)
                    d = l2.tile([128, C, 128], BF16, tag="d2")
                    nc.gpsimd.dma_gather(
                        d[:, :, :], s_tbl[:],
                        didx_t[:, base // 16:(base + C * 128) // 16],
                        num_idxs=C * 128, num_idxs_reg=C * 128,
                        elem_size=128, single_packet=False)

                    q = l2.tile([128, C, 1], F32, tag="q_2")
                    nc.vector.tensor_add(
                        q[:], g[:, :, OUT:OUT + 1],
                        d[:, :, 8:10].bitcast(F32))
                    q2 = l2.tile([128, C, 1], F32, tag="q2_2")
                    nc.vector.tensor_scalar_mul(q2[:], q[:], 0.01)
                    nc.vector.tensor_max(q2[:], q2[:], q[:])
                    num = l2.tile([128, C, 1], F32, tag="num2")
                    nc.scalar.activation(
                        num[:], q2[:], mybir.ActivationFunctionType.Exp)

                    m = l2m.tile([128, C, OUT + 1], F32, tag="m2")
                    nc.vector.tensor_tensor(
                        m[:, :, 0:OUT], g[:, :, 0:OUT],
                        num[:].to_broadcast((128, C, OUT)),
                        mybir.AluOpType.mult)
                    nc.vector.tensor_copy(m[:, :, OUT:OUT + 1], num[:])

                    acc = a2p[gi % 2]
                    for c0 in range(0, C, PIECE_CHUNKS):
                        cn = min(PIECE_CHUNKS, C - c0)
                        p0 = base + c0 * 128
                        nc.gpsimd.dma_scatter_add(
                            acc[:, 0:OUT + 1], m[:, c0:c0 + cn, :],
                            didx_t[:, p0 // 16:(p0 + cn * 128) // 16],
                            num_idxs=cn * 128, num_idxs_reg=cn * 128,
                            elem_size=OUT + 1, elem_step=128)

            # ---------------- D3: out = agg / numer -----------------------
            with tc.tile_pool(name="d3", bufs=3) as d3:
                for t in range(NT):
                    r0, r1 = t * 128, (t + 1) * 128
                    a0 = d3.tile([128, OUT + 1], F32, tag="b0")
                    a1t = d3.tile([128, OUT + 1], F32, tag="b1")
                    nc.sync.dma_start(a0[:], a2p[0][r0:r1, 0:OUT + 1])
                    nc.sync.dma_start(a1t[:], a2p[1][r0:r1, 0:OUT + 1])
                    msum = d3.tile([128, OUT + 1], F32, tag="bsum")
                    nc.vector.tensor_add(msum[:], a0[:], a1t[:])
                    nm = d3.tile([128, 1], F32, tag="bnm")
                    nc.vector.tensor_scalar_max(
                        nm[:], msum[:, OUT:OUT + 1], 1e-30)
                    rec = d3.tile([128, 1], F32, tag="brec")
                    nc.vector.reciprocal(rec[:], nm[:])
                    ot = d3.tile([128, OUT], F32, tag="ot")
                    nc.vector.tensor_scalar_mul(ot[:], msum[:, 0:OUT], rec[:])
                    nc.sync.dma_start(out_ext[r0:r1, :], ot[:])

    nc.compile()
    return nc


# ----------------------------------------------------------------- frontend

_CACHE = {}


def _run(h, src, dst, W1, a1, W2, a2, trace=False):
    struct, in_maps = preprocess(h, src, dst, W1, a1, W2, a2)
    key = (struct["N"], struct["E"], struct["total_pos"],
           tuple(struct["groups"]))
    if key not in _CACHE:
        _CACHE[key] = build(struct)
    nc = _CACHE[key]
    res = run_bass_kernel_spmd(nc, in_maps, core_ids=list(range(NCORES)),
                               trace=trace)
    npc = struct["npc"]
    out = np.concatenate(
        [res.results[c]["out"][:npc] for c in range(NCORES)], 0)
    return out.astype(np.float32), res


def kernel(h, src, dst, W1, a1, W2, a2):
    h = np.asarray(h, dtype=np.float32)
    src = np.asarray(src, dtype=np.int32)
    dst = np.asarray(dst, dtype=np.int32)
    W1 = np.asarray(W1, dtype=np.float32)
    a1 = np.asarray(a1, dtype=np.float32)
    W2 = np.asarray(W2, dtype=np.float32)
    a2 = np.asarray(a2, dtype=np.float32)
    out, _ = _run(h, src, dst, W1, a1, W2, a2, trace=False)
    return out


# revision 6
# speedup vs baseline: 1.4094x; 1.4094x over previous
"""2-layer multi-head GAT on 8 TRN2 NeuronCores (Bass/Tile), v2.

Sharding: destination-node blocks. Core i owns nodes [i*NPC, (i+1)*NPC) and
all edges whose dst lands there, so edge softmax + aggregation are fully
core-local (no all-reduce). z-tables are replicated via two small AllGathers.

Edge phase per layer, per 128-node window (edges sorted by dst, chunk-padded
so every 128-edge chunk stays within one window):
  - dma_gather of per-src table rows (z | s_src packed, bf16 for layer 1)
  - dma_gather of per-dst score rows (s_dst) from a core-local table
  - numerator n_e = exp(leakyrelu(s_src+s_dst)) on DVE/ACT; padding edges get
    s_dst = -1e30 via a dummy table row so n_e = 0
  - one-hot O[e, j] = (dst_local(e) == j) built on DVE from iota/wloc
  - TensorE aggregation into PSUM: agg[j, :] += sum_e O[e, j] * [n_e*z_e|n_e]
  - window flush feeds the next dense stage directly (no HBM accumulators,
    no scatter-adds).
"""
import sys
sys.path.insert(0, "/opt/trn_rl_repo")

import numpy as np

import concourse.bass as bass
import concourse.bacc as bacc
import concourse.tile as tile
import concourse.mybir as mybir
from concourse.bass_utils import run_bass_kernel_spmd
from concourse.masks import make_identity

F32 = mybir.dt.float32
BF16 = mybir.dt.bfloat16
I16 = mybir.dt.int16

NCORES = 8
HALF = 32768           # int16 gather index split
NEG = -1.0e30          # dst-score of dummy rows -> numerator exactly 0


def _round_up(x, m):
    return (x + m - 1) // m * m


# ----------------------------------------------------------------- host prep

def preprocess(h, src, dst, W1, a1, W2, a2):
    N, IN_DIM = h.shape
    HEADS, _, HID = W1.shape
    OUT = W2.shape[1]
    npc = N // NCORES
    rows = _round_up(npc + 1, 128)
    dummy = npc
    NW = rows // 128

    # weight folding (weights-only algebra)
    w1cat = np.transpose(W1, (1, 0, 2)).reshape(IN_DIM, HEADS * HID)
    w1s = np.stack([W1[hh] @ a1[hh, :HID] for hh in range(HEADS)], 1)
    w1d = np.stack([W1[hh] @ a1[hh, HID:] for hh in range(HEADS)], 1)
    wc1 = np.concatenate([w1cat, w1s, w1d], 1).astype(np.float32)
    wc2 = np.concatenate([W2, (W2 @ a2[:OUT])[:, None],
                          (W2 @ a2[OUT:])[:, None]], 1).astype(np.float32)

    core_of = dst // npc
    gsrc_all = (src // npc) * rows + (src % npc)

    pc = []
    for c in range(NCORES):
        m = core_of == c
        dstl = (dst[m] - c * npc).astype(np.int64)
        gsrc = gsrc_all[m].astype(np.int64)
        pc.append((dstl, gsrc))

    # per (core, window, half) counts -> static chunk structure
    low_cnt = np.zeros((NCORES, NW), dtype=np.int64)
    high_cnt = np.zeros((NCORES, NW), dtype=np.int64)
    for c in range(NCORES):
        dstl, gsrc = pc[c]
        w = dstl // 128
        hi = gsrc >= HALF
        np.add.at(low_cnt[c], w[~hi], 1)
        np.add.at(high_cnt[c], w[hi], 1)
    KL = np.maximum(1, np.ceil(low_cnt.max(0) / 128.0).astype(np.int64))
    KH = np.ceil(high_cnt.max(0) / 128.0).astype(np.int64)
    chunks = KL + KH
    bases = (np.concatenate([[0], np.cumsum(chunks)]) * 128).astype(np.int64)
    total_pos = int(bases[-1])

    eidx = np.zeros((NCORES, total_pos), dtype=np.int16)
    didx = np.full((NCORES, total_pos), dummy, dtype=np.int16)
    for c in range(NCORES):
        dstl, gsrc = pc[c]
        w = dstl // 128
        hi = (gsrc >= HALF).astype(np.int64)
        key = w * 2 + hi
        order = np.argsort(key, kind="stable")
        ks = key[order]
        new = np.ones(len(ks), dtype=bool)
        new[1:] = ks[1:] != ks[:-1]
        starts = np.flatnonzero(new)
        lens = np.diff(np.append(starts, len(ks)))
        within = np.arange(len(ks)) - np.repeat(starts, lens)
        w_o, h_o = w[order], hi[order]
        pos = bases[w_o] + h_o * KL[w_o] * 128 + within
        eidx[c, pos] = (gsrc[order] - h_o * HALF).astype(np.int16)
        didx[c, pos] = dstl[order].astype(np.int16)

    # wloc: dst-local offset within the chunk's window, [128, total/128] f32
    win_of_chunk = np.repeat(np.arange(NW), chunks)
    win_of_pos = np.repeat(win_of_chunk, 128)
    wloc = didx.astype(np.float32) - (win_of_pos * 128.0)[None, :]
    wloc_t = np.ascontiguousarray(
        wloc.reshape(NCORES, total_pos // 128, 128).transpose(0, 2, 1))

    windows = [(int(bases[w]), int(KL[w]), int(KH[w])) for w in range(NW)]
    struct = dict(
        N=N, E=src.shape[0], IN_DIM=IN_DIM, HEADS=HEADS, HID=HID, OUT=OUT,
        npc=npc, rows=rows, total_pos=total_pos, windows=windows,
    )

    def idx_tile(a):
        t = a.reshape(-1, 16).T.copy()
        return np.concatenate([t, t], 0)

    lo = npc - (NW - 1) * 128
    dmask_host = np.zeros((128, 4), dtype=np.float32)
    dmask_host[lo:, :] = NEG
    iota_pf = np.tile(np.arange(128, dtype=np.float32)[None, :], (128, 1))

    in_maps = []
    for c in range(NCORES):
        hs = np.zeros((rows, IN_DIM), dtype=np.float32)
        hs[:npc] = h[c * npc:(c + 1) * npc]
        in_maps.append({
            "h": hs,
            "eidx": idx_tile(eidx[c]),
            "didx": idx_tile(didx[c]),
            "wloc": wloc_t[c],
            "iota": iota_pf,
            "wc1": wc1,
            "wc2": wc2,
            "dmask": dmask_host,
        })
    return struct, in_maps


# --------------------------------------------------------------- bass graph

def build(s):
    npc, rows, total_pos = s["npc"], s["rows"], s["total_pos"]
    windows = s["windows"]
    IN_DIM, HEADS, HID, OUT = s["IN_DIM"], s["HEADS"], s["HID"], s["OUT"]
    ZC = HEADS * HID
    NW = rows // 128

    nc = bacc.Bacc("TRN2", target_bir_lowering=False, debug=False,
                   num_devices=NCORES)

    h_in = nc.dram_tensor("h", [rows, IN_DIM], F32, kind="ExternalInput")
    eidx_in = nc.dram_tensor("eidx", [32, total_pos // 16], I16,
                             kind="ExternalInput")
    didx_in = nc.dram_tensor("didx", [32, total_pos // 16], I16,
                             kind="ExternalInput")
    wloc_in = nc.dram_tensor("wloc", [128, total_pos // 128], F32,
                             kind="ExternalInput")
    iota_in = nc.dram_tensor("iota", [128, 128], F32, kind="ExternalInput")
    wc1_in = nc.dram_tensor("wc1", [IN_DIM, ZC + 8], F32, kind="ExternalInput")
    wc2_in = nc.dram_tensor("wc2", [ZC, OUT + 2], F32, kind="ExternalInput")
    dmask_in = nc.dram_tensor("dmask", [128, 4], F32, kind="ExternalInput")
    out_ext = nc.dram_tensor("out", [rows, OUT], F32, kind="ExternalOutput")

    with tile.TileContext(nc) as tc:
        with (
            tc.tile_pool(name="dram", bufs=1, space="DRAM") as dram,
            tc.tile_pool(name="const", bufs=1) as const,
            tc.tile_pool(name="psum_c", bufs=2, space="PSUM") as psum_c,
        ):
            t1_loc = dram.tile([rows, 384], BF16)
            t1_full = dram.tile([NCORES * rows, 384], BF16)
            t2_loc = dram.tile([rows, 128], F32)
            t2_full = dram.tile([NCORES * rows, 128], F32)
            s_tbl = dram.tile([rows, 128], BF16)

            ident = const.tile([128, 128], F32)
            make_identity(nc, ident[:])
            wc1_t = const.tile([IN_DIM, ZC + 8], F32)
            nc.sync.dma_start(wc1_t[:], wc1_in[:])
            wc2a = const.tile([128, OUT + 2], F32)
            wc2b = const.tile([128, OUT + 2], F32)
            nc.sync.dma_start(wc2a[:], wc2_in[0:128, :])
            nc.sync.dma_start(wc2b[:], wc2_in[128:256, :])
            dmask = const.tile([128, 4], F32)
            nc.sync.dma_start(dmask[:], dmask_in[:])
            iota = const.tile([128, 128], F32)
            nc.sync.dma_start(iota[:], iota_in[:])
            eidx_t = const.tile([32, total_pos // 16], I16)
            didx_t = const.tile([32, total_pos // 16], I16)
            nc.sync.dma_start(eidx_t[:], eidx_in[:])
            nc.sync.dma_start(didx_t[:], didx_in[:])
            wloc_t = const.tile([128, total_pos // 128], F32)
            nc.sync.dma_start(wloc_t[:], wloc_in[:])

            # ---------------- D1: z1 | s_src1 | s_dst1 -> T1, S ----------
            with tc.tile_pool(name="d1", bufs=3) as d1:
                for t in range(NW):
                    ht = d1.tile([128, IN_DIM], F32, tag="ht")
                    nc.sync.dma_start(ht[:], h_in[t * 128:(t + 1) * 128, :])
                    hT_ps = psum_c.tile([128, 128], F32, tag="tp")
                    nc.tensor.transpose(hT_ps[:], ht[:], ident[:])
                    hT = d1.tile([128, 128], F32, tag="hT")
                    nc.vector.tensor_copy(hT[:], hT_ps[:])
                    zps = psum_c.tile([128, ZC + 8], F32, tag="zp")
                    nc.tensor.matmul(zps[:], hT[:], wc1_t[:])

                    t1t = d1.tile([128, 384], BF16, tag="t1t")
                    nc.vector.tensor_copy(t1t[:, 0:ZC], zps[:, 0:ZC])
                    nc.vector.tensor_copy(
                        t1t[:, ZC:ZC + 16].bitcast(F32), zps[:, ZC:ZC + 8])
                    st = d1.tile([128, 128], BF16, tag="st")
                    if t == NW - 1:
                        nc.vector.tensor_add(
                            st[:, 0:8].bitcast(F32),
                            zps[:, ZC + 4:ZC + 8], dmask[:])
                    else:
                        nc.vector.tensor_copy(
                            st[:, 0:8].bitcast(F32), zps[:, ZC + 4:ZC + 8])
                    nc.sync.dma_start(
                        t1_loc[t * 128:(t + 1) * 128, :], t1t[:])
                    nc.sync.dma_start(
                        s_tbl[t * 128:(t + 1) * 128, :], st[:])

            nc.gpsimd.collective_compute(
                "AllGather", mybir.AluOpType.bypass,
                replica_groups=[list(range(NCORES))],
                ins=[t1_loc.opt()], outs=[t1_full.opt()],
            )

            # ------- L1 edge phase + fused D2, per 128-node window --------
            with (
                tc.tile_pool(name="l1", bufs=3) as l1,
                tc.tile_pool(name="l1m", bufs=3) as l1m,
                tc.tile_pool(name="d2", bufs=3) as d2,
                tc.tile_pool(name="psum_w", bufs=2, space="PSUM") as psum_w,
            ):
                for wi, (base, KLw, KHw) in enumerate(windows):
                    C = KLw + KHw
                    bc = base // 128
                    g = l1.tile([128, C, 384], BF16, tag="g")
                    nc.gpsimd.dma_gather(
                        g[:, 0:KLw, :], t1_full[:],
                        eidx_t[:, base // 16:(base + KLw * 128) // 16],
                        num_idxs=KLw * 128, num_idxs_reg=KLw * 128,
                        elem_size=384, single_packet=False)
                    if KHw:
                        b2 = base + KLw * 128
                        nc.gpsimd.dma_gather(
                            g[:, KLw:C, :], t1_full[HALF:, :],
                            eidx_t[:, b2 // 16:(b2 + KHw * 128) // 16],
                            num_idxs=KHw * 128, num_idxs_reg=KHw * 128,
                            elem_size=384, single_packet=False)
                    d = l1.tile([128, C, 128], BF16, tag="d")
                    nc.gpsimd.dma_gather(
                        d[:, :, :], s_tbl[:],
                        didx_t[:, base // 16:(base + C * 128) // 16],
                        num_idxs=C * 128, num_idxs_reg=C * 128,
                        elem_size=128, single_packet=False)

                    q = l1.tile([128, C, HEADS], F32, tag="q")
                    nc.vector.tensor_add(
                        q[:], g[:, :, ZC:ZC + 8].bitcast(F32),
                        d[:, :, 0:8].bitcast(F32))
                    q2 = l1.tile([128, C, HEADS], F32, tag="q2")
                    nc.vector.tensor_scalar_mul(q2[:], q[:], 0.01)
                    nc.vector.tensor_max(q2[:], q2[:], q[:])
                    num = l1.tile([128, C, HEADS], F32, tag="num")
                    nc.scalar.activation(
                        num[:], q2[:], mybir.ActivationFunctionType.Exp)

                    m = l1m.tile([128, C, ZC + HEADS], BF16, tag="m")
                    nc.vector.tensor_tensor(
                        m[:, :, 0:ZC].rearrange(
                            "p c (h x) -> p c h x", h=HEADS),
                        g[:, :, 0:ZC].rearrange(
                            "p c (h x) -> p c h x", h=HEADS),
                        num[:, :, :, None].to_broadcast((128, C, HEADS, HID)),
                        mybir.AluOpType.mult)
                    nc.vector.tensor_copy(m[:, :, ZC:ZC + HEADS], num[:])

                    o = l1m.tile([128, C, 128], BF16, tag="o")
                    nc.vector.tensor_tensor(
                        o[:],
                        wloc_t[:, bc:bc + C, None].to_broadcast((128, C, 128)),
                        iota[:, None, :].to_broadcast((128, C, 128)),
                        mybir.AluOpType.is_equal)

                    agg = psum_w.tile([128, ZC + HEADS], F32, tag="agg")
                    for cc in range(C):
                        nc.tensor.matmul(
                            agg[:], o[:, cc, :], m[:, cc, :],
                            start=(cc == 0), stop=(cc == C - 1))

                    # fused D2 for this window's 128 nodes
                    msum = d2.tile([128, ZC + HEADS], F32, tag="msum")
                    nc.vector.tensor_copy(msum[:], agg[:])
                    nm = d2.tile([128, HEADS], F32, tag="nm")
                    nc.vector.tensor_scalar_max(
                        nm[:], msum[:, ZC:ZC + HEADS], 1e-30)
                    rec = d2.tile([128, HEADS], F32, tag="rec")
                    nc.vector.reciprocal(rec[:], nm[:])
                    h1 = d2.tile([128, ZC], F32, tag="h1")
                    nc.vector.tensor_tensor(
                        h1[:].rearrange("p (h x) -> p h x", h=HEADS),
                        msum[:, 0:ZC].rearrange("p (h x) -> p h x", h=HEADS),
                        rec[:, :, None].to_broadcast((128, HEADS, HID)),
                        mybir.AluOpType.mult)
                    relu = d2.tile([128, ZC], F32, tag="relu")
                    nc.scalar.activation(
                        relu[:], h1[:], mybir.ActivationFunctionType.Relu)
                    xm = d2.tile([128, ZC], F32, tag="xm")
                    nc.vector.tensor_scalar_min(xm[:], h1[:], 0.0)
                    ex = d2.tile([128, ZC], F32, tag="ex")
                    nc.scalar.activation(
                        ex[:], xm[:], mybir.ActivationFunctionType.Exp)
                    h1e = d2.tile([128, ZC], F32, tag="h1e")
                    nc.vector.tensor_add(h1e[:], relu[:], ex[:])
                    nc.vector.tensor_scalar_add(h1e[:], h1e[:], -1.0)

                    z2ps = psum_c.tile([128, OUT + 2], F32, tag="z2p")
                    for kk in range(2):
                        tp = psum_c.tile([128, 128], F32, tag="tp")
                        nc.tensor.transpose(
                            tp[:], h1e[:, kk * 128:(kk + 1) * 128], ident[:])
                        hT2 = d2.tile([128, 128], F32, tag="hT2")
                        nc.vector.tensor_copy(hT2[:], tp[:])
                        nc.tensor.matmul(
                            z2ps[:], hT2[:], wc2a[:] if kk == 0 else wc2b[:],
                            start=(kk == 0), stop=(kk == 1))

                    r0, r1 = wi * 128, (wi + 1) * 128
                    t2t = d2.tile([128, 128], F32, tag="t2t")
                    nc.vector.tensor_copy(t2t[:, 0:OUT + 1], z2ps[:, 0:OUT + 1])
                    st2 = d2.tile([128, 2], BF16, tag="st2")
                    if wi == NW - 1:
                        nc.vector.tensor_add(
                            st2[:].bitcast(F32),
                            z2ps[:, OUT + 1:OUT + 2], dmask[:, 0:1])
                    else:
                        nc.vector.tensor_copy(
                            st2[:].bitcast(F32), z2ps[:, OUT + 1:OUT + 2])
                    nc.sync.dma_start(t2_loc[r0:r1, :], t2t[:])
                    nc.sync.dma_start(s_tbl[r0:r1, 8:10], st2[:])

            nc.gpsimd.collective_compute(
                "AllGather", mybir.AluOpType.bypass,
                replica_groups=[list(range(NCORES))],
                ins=[t2_loc.opt()], outs=[t2_full.opt()],
            )

            # ------- L2 edge phase + fused D3, per window -----------------
            with (
                tc.tile_pool(name="l2", bufs=3) as l2,
                tc.tile_pool(name="l2m", bufs=3) as l2m,
                tc.tile_pool(name="psum_w2", bufs=2, space="PSUM") as psum_w2,
            ):
                for wi, (base, KLw, KHw) in enumerate(windows):
                    C = KLw + KHw
                    bc = base // 128
                    g = l2.tile([128, C, 128], F32, tag="g2")
                    nc.gpsimd.dma_gather(
                        g[:, 0:KLw, :], t2_full[:],
                        eidx_t[:, base // 16:(base + KLw * 128) // 16],
                        num_idxs=KLw * 128, num_idxs_reg=KLw * 128,
                        elem_size=128, single_packet=False)
                    if KHw:
                        b2 = base + KLw * 128
                        nc.gpsimd.dma_gather(
                            g[:, KLw:C, :], t2_full[HALF:, :],
                            eidx_t[:, b2 // 16:(b2 + KHw * 128) // 16],
                            num_idxs=KHw * 128, num_idxs_reg=KHw * 128,
                            elem_size=128, single_packet=False)
                    d = l2.tile([128, C, 128], BF16, tag="d2")
                    nc.gpsimd.dma_gather(
                        d[:, :, :], s_tbl[:],
                        didx_t[:, base // 16:(base + C * 128) // 16],
                        num_idxs=C * 128, num_idxs_reg=C * 128,
                        elem_size=128, single_packet=False)

                    q = l2.tile([128, C, 1], F32, tag="q_2")
                    nc.vector.tensor_add(
                        q[:], g[:, :, OUT:OUT + 1],
                        d[:, :, 8:10].bitcast(F32))
                    q2 = l2.tile([128, C, 1], F32, tag="q2_2")
                    nc.vector.tensor_scalar_mul(q2[:], q[:], 0.01)
                    nc.vector.tensor_max(q2[:], q2[:], q[:])
                    num = l2.tile([128, C, 1], F32, tag="num2")
                    nc.scalar.activation(
                        num[:], q2[:], mybir.ActivationFunctionType.Exp)

                    m = l2m.tile([128, C, OUT + 1], BF16, tag="m2")
                    nc.vector.tensor_tensor(
                        m[:, :, 0:OUT], g[:, :, 0:OUT],
                        num[:].to_broadcast((128, C, OUT)),
                        mybir.AluOpType.mult)
                    nc.vector.tensor_copy(m[:, :, OUT:OUT + 1], num[:])

                    o = l2m.tile([128, C, 128], BF16, tag="o2")
                    nc.vector.tensor_tensor(
                        o[:],
                        wloc_t[:, bc:bc + C, None].to_broadcast((128, C, 128)),
                        iota[:, None, :].to_broadcast((128, C, 128)),
                        mybir.AluOpType.is_equal)

                    agg = psum_w2.tile([128, OUT + 1], F32, tag="agg2")
                    for cc in range(C):
                        nc.tensor.matmul(
                            agg[:], o[:, cc, :], m[:, cc, :],
                            start=(cc == 0), stop=(cc == C - 1))

                    msum = l2.tile([128, OUT + 1], F32, tag="bsum")
                    nc.vector.tensor_copy(msum[:], agg[:])
                    nm = l2.tile([128, 1], F32, tag="bnm")
                    nc.vector.tensor_scalar_max(
                        nm[:], msum[:, OUT:OUT + 1], 1e-30)
                    rec = l2.tile([128, 1], F32, tag="brec")
                    nc.vector.reciprocal(rec[:], nm[:])
                    ot = l2.tile([128, OUT], F32, tag="ot")
                    nc.vector.tensor_scalar_mul(ot[:], msum[:, 0:OUT], rec[:])
                    nc.sync.dma_start(
                        out_ext[wi * 128:(wi + 1) * 128, :], ot[:])

    nc.compile()
    return nc


# ----------------------------------------------------------------- frontend

_CACHE = {}


def _run(h, src, dst, W1, a1, W2, a2, trace=False):
    struct, in_maps = preprocess(h, src, dst, W1, a1, W2, a2)
    key = (struct["N"], struct["E"], struct["total_pos"],
           tuple(struct["windows"]))
    if key not in _CACHE:
        _CACHE[key] = build(struct)
    nc = _CACHE[key]
    res = run_bass_kernel_spmd(nc, in_maps, core_ids=list(range(NCORES)),
                               trace=trace)
    npc = struct["npc"]
    out = np.concatenate(
        [res.results[c]["out"][:npc] for c in range(NCORES)], 0)
    return out.astype(np.float32), res


def kernel(h, src, dst, W1, a1, W2, a2):
    h = np.asarray(h, dtype=np.float32)
    src = np.asarray(src, dtype=np.int32)
    dst = np.asarray(dst, dtype=np.int32)
    W1 = np.asarray(W1, dtype=np.float32)
    a1 = np.asarray(a1, dtype=np.float32)
    W2 = np.asarray(W2, dtype=np.float32)
    a2 = np.asarray(a2, dtype=np.float32)
    out, _ = _run(h, src, dst, W1, a1, W2, a2, trace=False)
    return out


# revision 7
# speedup vs baseline: 1.8291x; 1.2978x over previous
"""2-layer multi-head GAT on 8 TRN2 NeuronCores (Bass/Tile), v2.

Sharding: destination-node blocks. Core i owns nodes [i*NPC, (i+1)*NPC) and
all edges whose dst lands there, so edge softmax + aggregation are fully
core-local (no all-reduce). z-tables are replicated via two small AllGathers.

Edge phase per layer, per 128-node window (edges sorted by dst, chunk-padded
so every 128-edge chunk stays within one window):
  - dma_gather of per-src table rows (z | s_src packed, bf16 for layer 1)
  - dma_gather of per-dst score rows (s_dst) from a core-local table
  - numerator n_e = exp(leakyrelu(s_src+s_dst)) on DVE/ACT; padding edges get
    s_dst = -1e30 via a dummy table row so n_e = 0
  - one-hot O[e, j] = (dst_local(e) == j) built on DVE from iota/wloc
  - TensorE aggregation into PSUM: agg[j, :] += sum_e O[e, j] * [n_e*z_e|n_e]
  - window flush feeds the next dense stage directly (no HBM accumulators,
    no scatter-adds).
"""
import sys
sys.path.insert(0, "/opt/trn_rl_repo")

import numpy as np

import concourse.bass as bass
import concourse.bacc as bacc
import concourse.tile as tile
import concourse.mybir as mybir
from concourse.bass_utils import run_bass_kernel_spmd
from concourse.masks import make_identity

F32 = mybir.dt.float32
BF16 = mybir.dt.bfloat16
I16 = mybir.dt.int16

NCORES = 8
HALF = 32768           # int16 gather index split
NEG = -1.0e30          # dst-score of dummy rows -> numerator exactly 0


def _round_up(x, m):
    return (x + m - 1) // m * m


# ----------------------------------------------------------------- host prep

def preprocess(h, src, dst, W1, a1, W2, a2):
    N, IN_DIM = h.shape
    HEADS, _, HID = W1.shape
    OUT = W2.shape[1]
    npc = N // NCORES
    rows = _round_up(npc + 1, 128)
    dummy = npc
    NW = rows // 128

    # weight folding (weights-only algebra)
    w1cat = np.transpose(W1, (1, 0, 2)).reshape(IN_DIM, HEADS * HID)
    w1s = np.stack([W1[hh] @ a1[hh, :HID] for hh in range(HEADS)], 1)
    w1d = np.stack([W1[hh] @ a1[hh, HID:] for hh in range(HEADS)], 1)
    wc1 = np.concatenate([w1cat, w1s, w1d], 1).astype(np.float32)
    wc2 = np.concatenate([W2, (W2 @ a2[:OUT])[:, None],
                          (W2 @ a2[OUT:])[:, None]], 1).astype(np.float32)

    core_of = dst // npc
    gsrc_all = (src // npc) * rows + (src % npc)

    pc = []
    for c in range(NCORES):
        m = core_of == c
        dstl = (dst[m] - c * npc).astype(np.int64)
        gsrc = gsrc_all[m].astype(np.int64)
        pc.append((dstl, gsrc))

    # per (core, window, half) counts -> static chunk structure
    low_cnt = np.zeros((NCORES, NW), dtype=np.int64)
    high_cnt = np.zeros((NCORES, NW), dtype=np.int64)
    for c in range(NCORES):
        dstl, gsrc = pc[c]
        w = dstl // 128
        hi = gsrc >= HALF
        np.add.at(low_cnt[c], w[~hi], 1)
        np.add.at(high_cnt[c], w[hi], 1)
    KL = np.maximum(1, np.ceil(low_cnt.max(0) / 128.0).astype(np.int64))
    KH = np.ceil(high_cnt.max(0) / 128.0).astype(np.int64)
    chunks = KL + KH
    bases = (np.concatenate([[0], np.cumsum(chunks)]) * 128).astype(np.int64)
    total_pos = int(bases[-1])

    eidx = np.zeros((NCORES, total_pos), dtype=np.int16)
    didx = np.full((NCORES, total_pos), dummy, dtype=np.int16)
    for c in range(NCORES):
        dstl, gsrc = pc[c]
        w = dstl // 128
        hi = (gsrc >= HALF).astype(np.int64)
        key = w * 2 + hi
        order = np.argsort(key, kind="stable")
        ks = key[order]
        new = np.ones(len(ks), dtype=bool)
        new[1:] = ks[1:] != ks[:-1]
        starts = np.flatnonzero(new)
        lens = np.diff(np.append(starts, len(ks)))
        within = np.arange(len(ks)) - np.repeat(starts, lens)
        w_o, h_o = w[order], hi[order]
        pos = bases[w_o] + h_o * KL[w_o] * 128 + within
        eidx[c, pos] = (gsrc[order] - h_o * HALF).astype(np.int16)
        didx[c, pos] = dstl[order].astype(np.int16)

    # wloc: dst-local offset within the chunk's window, [128, total/128] f32
    win_of_chunk = np.repeat(np.arange(NW), chunks)
    win_of_pos = np.repeat(win_of_chunk, 128)
    wloc = didx.astype(np.float32) - (win_of_pos * 128.0)[None, :]
    wloc_t = np.ascontiguousarray(
        wloc.reshape(NCORES, total_pos // 128, 128).transpose(0, 2, 1))

    windows = [(int(bases[w]), int(KL[w]), int(KH[w])) for w in range(NW)]
    struct = dict(
        N=N, E=src.shape[0], IN_DIM=IN_DIM, HEADS=HEADS, HID=HID, OUT=OUT,
        npc=npc, rows=rows, total_pos=total_pos, windows=windows,
    )

    def idx_tile(a):
        t = a.reshape(-1, 16).T.copy()
        return np.concatenate([t] * 8, 0)

    lo = npc - (NW - 1) * 128
    dmask_host = np.zeros((128, 4), dtype=np.float32)
    dmask_host[lo:, :] = NEG
    iota_pf = np.tile(np.arange(128, dtype=np.float32)[None, :], (128, 1))

    in_maps = []
    for c in range(NCORES):
        hs = np.zeros((rows, IN_DIM), dtype=np.float32)
        hs[:npc] = h[c * npc:(c + 1) * npc]
        in_maps.append({
            "h": hs,
            "eidx": idx_tile(eidx[c]),
            "didx": idx_tile(didx[c]),
            "wloc": wloc_t[c],
            "iota": iota_pf,
            "wc1": wc1,
            "wc2": wc2,
            "dmask": dmask_host,
        })
    return struct, in_maps


# --------------------------------------------------------------- bass graph

def build(s):
    npc, rows, total_pos = s["npc"], s["rows"], s["total_pos"]
    windows = s["windows"]
    IN_DIM, HEADS, HID, OUT = s["IN_DIM"], s["HEADS"], s["HID"], s["OUT"]
    ZC = HEADS * HID
    NW = rows // 128

    nc = bacc.Bacc("TRN2", target_bir_lowering=False, debug=False,
                   num_devices=NCORES, num_swdge_queues=4)

    h_in = nc.dram_tensor("h", [rows, IN_DIM], F32, kind="ExternalInput")
    eidx_in = nc.dram_tensor("eidx", [128, total_pos // 16], I16,
                             kind="ExternalInput")
    didx_in = nc.dram_tensor("didx", [128, total_pos // 16], I16,
                             kind="ExternalInput")
    wloc_in = nc.dram_tensor("wloc", [128, total_pos // 128], F32,
                             kind="ExternalInput")
    iota_in = nc.dram_tensor("iota", [128, 128], F32, kind="ExternalInput")
    wc1_in = nc.dram_tensor("wc1", [IN_DIM, ZC + 8], F32, kind="ExternalInput")
    wc2_in = nc.dram_tensor("wc2", [ZC, OUT + 2], F32, kind="ExternalInput")
    dmask_in = nc.dram_tensor("dmask", [128, 4], F32, kind="ExternalInput")
    out_ext = nc.dram_tensor("out", [rows, OUT], F32, kind="ExternalOutput")

    with tile.TileContext(nc) as tc:
        with (
            tc.tile_pool(name="dram", bufs=1, space="DRAM") as dram,
            tc.tile_pool(name="const", bufs=1) as const,
            tc.tile_pool(name="psum_c", bufs=2, space="PSUM") as psum_c,
        ):
            t1_loc = dram.tile([rows, 384], BF16)
            t1_full = nc.dram_tensor("t1_full_sh", [NCORES * rows, 384],
                                     BF16, kind="Internal",
                                     addr_space="Shared").ap()
            t2_loc = dram.tile([rows, 128], F32)
            t2_full = nc.dram_tensor("t2_full_sh", [NCORES * rows, 128],
                                     F32, kind="Internal",
                                     addr_space="Shared").ap()
            s_tbl = dram.tile([rows, 128], BF16)

            ident = const.tile([128, 128], F32)
            make_identity(nc, ident[:])
            wc1_t = const.tile([IN_DIM, ZC + 8], F32)
            nc.sync.dma_start(wc1_t[:], wc1_in[:])
            wc2a = const.tile([128, OUT + 2], F32)
            wc2b = const.tile([128, OUT + 2], F32)
            nc.sync.dma_start(wc2a[:], wc2_in[0:128, :])
            nc.sync.dma_start(wc2b[:], wc2_in[128:256, :])
            dmask = const.tile([128, 4], F32)
            nc.sync.dma_start(dmask[:], dmask_in[:])
            iota = const.tile([128, 128], F32)
            nc.sync.dma_start(iota[:], iota_in[:])
            eidx_t = const.tile([128, total_pos // 16], I16)
            didx_t = const.tile([128, total_pos // 16], I16)
            nc.sync.dma_start(eidx_t[:], eidx_in[:])
            nc.sync.dma_start(didx_t[:], didx_in[:])
            wloc_t = const.tile([128, total_pos // 128], F32)
            nc.sync.dma_start(wloc_t[:], wloc_in[:])

            # ---------------- D1: z1 | s_src1 | s_dst1 -> T1, S ----------
            with tc.tile_pool(name="d1", bufs=3) as d1:
                for t in range(NW):
                    ht = d1.tile([128, IN_DIM], F32, tag="ht")
                    nc.sync.dma_start(ht[:], h_in[t * 128:(t + 1) * 128, :])
                    hT_ps = psum_c.tile([128, 128], F32, tag="tp")
                    nc.tensor.transpose(hT_ps[:], ht[:], ident[:])
                    hT = d1.tile([128, 128], F32, tag="hT")
                    nc.vector.tensor_copy(hT[:], hT_ps[:])
                    zps = psum_c.tile([128, ZC + 8], F32, tag="zp")
                    nc.tensor.matmul(zps[:], hT[:], wc1_t[:])

                    t1t = d1.tile([128, 384], BF16, tag="t1t")
                    nc.vector.tensor_copy(t1t[:, 0:ZC], zps[:, 0:ZC])
                    nc.vector.tensor_copy(
                        t1t[:, ZC:ZC + 16].bitcast(F32), zps[:, ZC:ZC + 8])
                    st = d1.tile([128, 128], BF16, tag="st")
                    if t == NW - 1:
                        nc.vector.tensor_add(
                            st[:, 0:8].bitcast(F32),
                            zps[:, ZC + 4:ZC + 8], dmask[:])
                    else:
                        nc.vector.tensor_copy(
                            st[:, 0:8].bitcast(F32), zps[:, ZC + 4:ZC + 8])
                    nc.sync.dma_start(
                        t1_loc[t * 128:(t + 1) * 128, :], t1t[:])
                    nc.sync.dma_start(
                        s_tbl[t * 128:(t + 1) * 128, :], st[:])

            nc.gpsimd.collective_compute(
                "AllGather", mybir.AluOpType.bypass,
                replica_groups=[list(range(NCORES))],
                ins=[t1_loc.opt()], outs=[t1_full[:].opt()],
            )

            # ------- L1 edge phase + fused D2, per 128-node window --------
            with (
                tc.tile_pool(name="l1", bufs=3) as l1,
                tc.tile_pool(name="l1m", bufs=3) as l1m,
                tc.tile_pool(name="d2", bufs=3) as d2,
                tc.tile_pool(name="psum_w", bufs=2, space="PSUM") as psum_w,
            ):
                qc = 0
                for wi, (base, KLw, KHw) in enumerate(windows):
                    C = KLw + KHw
                    bc = base // 128
                    g = l1.tile([128, C, 384], BF16, tag="g")
                    nc.gpsimd.dma_gather(
                        g[:, 0:KLw, :], t1_full[:],
                        eidx_t[:, base // 16:(base + KLw * 128) // 16],
                        num_idxs=KLw * 128, num_idxs_reg=KLw * 128,
                        elem_size=384, single_packet=False,
                        queue_num=qc % 4); qc += 1
                    if KHw:
                        b2 = base + KLw * 128
                        nc.gpsimd.dma_gather(
                            g[:, KLw:C, :], t1_full[HALF:, :],
                            eidx_t[:, b2 // 16:(b2 + KHw * 128) // 16],
                            num_idxs=KHw * 128, num_idxs_reg=KHw * 128,
                            elem_size=384, single_packet=False,
                            queue_num=qc % 4); qc += 1
                    d = l1.tile([128, C, 128], BF16, tag="d")
                    nc.gpsimd.dma_gather(
                        d[:, :, :], s_tbl[:],
                        didx_t[:, base // 16:(base + C * 128) // 16],
                        num_idxs=C * 128, num_idxs_reg=C * 128,
                        elem_size=128, single_packet=False,
                        queue_num=qc % 4); qc += 1

                    q = l1.tile([128, C, HEADS], F32, tag="q")
                    nc.vector.tensor_add(
                        q[:], g[:, :, ZC:ZC + 8].bitcast(F32),
                        d[:, :, 0:8].bitcast(F32))
                    q2 = l1.tile([128, C, HEADS], F32, tag="q2")
                    nc.vector.tensor_scalar_mul(q2[:], q[:], 0.01)
                    nc.vector.tensor_max(q2[:], q2[:], q[:])
                    num = l1.tile([128, C, HEADS], F32, tag="num")
                    nc.scalar.activation(
                        num[:], q2[:], mybir.ActivationFunctionType.Exp)

                    m = l1m.tile([128, C, ZC + HEADS], BF16, tag="m")
                    nc.vector.tensor_tensor(
                        m[:, :, 0:ZC].rearrange(
                            "p c (h x) -> p c h x", h=HEADS),
                        g[:, :, 0:ZC].rearrange(
                            "p c (h x) -> p c h x", h=HEADS),
                        num[:, :, :, None].to_broadcast((128, C, HEADS, HID)),
                        mybir.AluOpType.mult)
                    nc.vector.tensor_copy(m[:, :, ZC:ZC + HEADS], num[:])

                    o = l1m.tile([128, C, 128], BF16, tag="o")
                    nc.vector.tensor_tensor(
                        o[:],
                        wloc_t[:, bc:bc + C, None].to_broadcast((128, C, 128)),
                        iota[:, None, :].to_broadcast((128, C, 128)),
                        mybir.AluOpType.is_equal)

                    agg = psum_w.tile([128, ZC + HEADS], F32, tag="agg")
                    for cc in range(C):
                        nc.tensor.matmul(
                            agg[:], o[:, cc, :], m[:, cc, :],
                            start=(cc == 0), stop=(cc == C - 1))

                    # fused D2 for this window's 128 nodes
                    msum = d2.tile([128, ZC + HEADS], F32, tag="msum")
                    nc.vector.tensor_copy(msum[:], agg[:])
                    nm = d2.tile([128, HEADS], F32, tag="nm")
                    nc.vector.tensor_scalar_max(
                        nm[:], msum[:, ZC:ZC + HEADS], 1e-30)
                    rec = d2.tile([128, HEADS], F32, tag="rec")
                    nc.vector.reciprocal(rec[:], nm[:])
                    h1 = d2.tile([128, ZC], F32, tag="h1")
                    nc.vector.tensor_tensor(
                        h1[:].rearrange("p (h x) -> p h x", h=HEADS),
                        msum[:, 0:ZC].rearrange("p (h x) -> p h x", h=HEADS),
                        rec[:, :, None].to_broadcast((128, HEADS, HID)),
                        mybir.AluOpType.mult)
                    relu = d2.tile([128, ZC], F32, tag="relu")
                    nc.scalar.activation(
                        relu[:], h1[:], mybir.ActivationFunctionType.Relu)
                    xm = d2.tile([128, ZC], F32, tag="xm")
                    nc.vector.tensor_scalar_min(xm[:], h1[:], 0.0)
                    ex = d2.tile([128, ZC], F32, tag="ex")
                    nc.scalar.activation(
                        ex[:], xm[:], mybir.ActivationFunctionType.Exp)
                    h1e = d2.tile([128, ZC], F32, tag="h1e")
                    nc.vector.tensor_add(h1e[:], relu[:], ex[:])
                    nc.vector.tensor_scalar_add(h1e[:], h1e[:], -1.0)

                    z2ps = psum_c.tile([128, OUT + 2], F32, tag="z2p")
                    for kk in range(2):
                        tp = psum_c.tile([128, 128], F32, tag="tp")
                        nc.tensor.transpose(
                            tp[:], h1e[:, kk * 128:(kk + 1) * 128], ident[:])
                        hT2 = d2.tile([128, 128], F32, tag="hT2")
                        nc.vector.tensor_copy(hT2[:], tp[:])
                        nc.tensor.matmul(
                            z2ps[:], hT2[:], wc2a[:] if kk == 0 else wc2b[:],
                            start=(kk == 0), stop=(kk == 1))

                    r0, r1 = wi * 128, (wi + 1) * 128
                    t2t = d2.tile([128, 128], F32, tag="t2t")
                    nc.vector.tensor_copy(t2t[:, 0:OUT + 1], z2ps[:, 0:OUT + 1])
                    st2 = d2.tile([128, 2], BF16, tag="st2")
                    if wi == NW - 1:
                        nc.vector.tensor_add(
                            st2[:].bitcast(F32),
                            z2ps[:, OUT + 1:OUT + 2], dmask[:, 0:1])
                    else:
                        nc.vector.tensor_copy(
                            st2[:].bitcast(F32), z2ps[:, OUT + 1:OUT + 2])
                    nc.sync.dma_start(t2_loc[r0:r1, :], t2t[:])
                    nc.sync.dma_start(s_tbl[r0:r1, 8:10], st2[:])

            nc.gpsimd.collective_compute(
                "AllGather", mybir.AluOpType.bypass,
                replica_groups=[list(range(NCORES))],
                ins=[t2_loc.opt()], outs=[t2_full[:].opt()],
            )

            # ------- L2 edge phase + fused D3, per window -----------------
            with (
                tc.tile_pool(name="l2", bufs=3) as l2,
                tc.tile_pool(name="l2m", bufs=3) as l2m,
                tc.tile_pool(name="psum_w2", bufs=2, space="PSUM") as psum_w2,
            ):
                qc = 0
                for wi, (base, KLw, KHw) in enumerate(windows):
                    C = KLw + KHw
                    bc = base // 128
                    g = l2.tile([128, C, 128], F32, tag="g2")
                    nc.gpsimd.dma_gather(
                        g[:, 0:KLw, :], t2_full[:],
                        eidx_t[:, base // 16:(base + KLw * 128) // 16],
                        num_idxs=KLw * 128, num_idxs_reg=KLw * 128,
                        elem_size=128, single_packet=False,
                        queue_num=qc % 4); qc += 1
                    if KHw:
                        b2 = base + KLw * 128
                        nc.gpsimd.dma_gather(
                            g[:, KLw:C, :], t2_full[HALF:, :],
                            eidx_t[:, b2 // 16:(b2 + KHw * 128) // 16],
                            num_idxs=KHw * 128, num_idxs_reg=KHw * 128,
                            elem_size=128, single_packet=False,
                            queue_num=qc % 4); qc += 1
                    d = l2.tile([128, C, 128], BF16, tag="d2")
                    nc.gpsimd.dma_gather(
                        d[:, :, :], s_tbl[:],
                        didx_t[:, base // 16:(base + C * 128) // 16],
                        num_idxs=C * 128, num_idxs_reg=C * 128,
                        elem_size=128, single_packet=False,
                        queue_num=qc % 4); qc += 1

                    q = l2.tile([128, C, 1], F32, tag="q_2")
                    nc.vector.tensor_add(
                        q[:], g[:, :, OUT:OUT + 1],
                        d[:, :, 8:10].bitcast(F32))
                    q2 = l2.tile([128, C, 1], F32, tag="q2_2")
                    nc.vector.tensor_scalar_mul(q2[:], q[:], 0.01)
                    nc.vector.tensor_max(q2[:], q2[:], q[:])
                    num = l2.tile([128, C, 1], F32, tag="num2")
                    nc.scalar.activation(
                        num[:], q2[:], mybir.ActivationFunctionType.Exp)

                    m = l2m.tile([128, C, OUT + 1], BF16, tag="m2")
                    nc.vector.tensor_tensor(
                        m[:, :, 0:OUT], g[:, :, 0:OUT],
                        num[:].to_broadcast((128, C, OUT)),
                        mybir.AluOpType.mult)
                    nc.vector.tensor_copy(m[:, :, OUT:OUT + 1], num[:])

                    o = l2m.tile([128, C, 128], BF16, tag="o2")
                    nc.vector.tensor_tensor(
                        o[:],
                        wloc_t[:, bc:bc + C, None].to_broadcast((128, C, 128)),
                        iota[:, None, :].to_broadcast((128, C, 128)),
                        mybir.AluOpType.is_equal)

                    agg = psum_w2.tile([128, OUT + 1], F32, tag="agg2")
                    for cc in range(C):
                        nc.tensor.matmul(
                            agg[:], o[:, cc, :], m[:, cc, :],
                            start=(cc == 0), stop=(cc == C - 1))

                    msum = l2.tile([128, OUT + 1], F32, tag="bsum")
                    nc.vector.tensor_copy(msum[:], agg[:])
                    nm = l2.tile([128, 1], F32, tag="bnm")
                    nc.vector.tensor_scalar_max(
                        nm[:], msum[:, OUT:OUT + 1], 1e-30)
                    rec = l2.tile([128, 1], F32, tag="brec")
                    nc.vector.reciprocal(rec[:], nm[:])
                    ot = l2.tile([128, OUT], F32, tag="ot")
                    nc.vector.tensor_scalar_mul(ot[:], msum[:, 0:OUT], rec[:])
                    nc.sync.dma_start(
                        out_ext[wi * 128:(wi + 1) * 128, :], ot[:])

    nc.compile()
    return nc


# ----------------------------------------------------------------- frontend

_CACHE = {}


def _run(h, src, dst, W1, a1, W2, a2, trace=False):
    struct, in_maps = preprocess(h, src, dst, W1, a1, W2, a2)
    key = (struct["N"], struct["E"], struct["total_pos"],
           tuple(struct["windows"]))
    if key not in _CACHE:
        _CACHE[key] = build(struct)
    nc = _CACHE[key]
    res = run_bass_kernel_spmd(nc, in_maps, core_ids=list(range(NCORES)),
                               trace=trace)
    npc = struct["npc"]
    out = np.concatenate(
        [res.results[c]["out"][:npc] for c in range(NCORES)], 0)
    return out.astype(np.float32), res


def kernel(h, src, dst, W1, a1, W2, a2):
    h = np.asarray(h, dtype=np.float32)
    src = np.asarray(src, dtype=np.int32)
    dst = np.asarray(dst, dtype=np.int32)
    W1 = np.asarray(W1, dtype=np.float32)
    a1 = np.asarray(a1, dtype=np.float32)
    W2 = np.asarray(W2, dtype=np.float32)
    a2 = np.asarray(a2, dtype=np.float32)
    out, _ = _run(h, src, dst, W1, a1, W2, a2, trace=False)
    return out
